# revision 11
# baseline (speedup 1.0000x reference)
"""GCN message-passing kernel for 8 trn2 NeuronCores (Bass/Tile).

Math (reference):
  x1 = relu(segsum(feat) @ W1 + b1)
  x2 = relu(segsum(x1) @ W2 + b2)
  out = relu(x2 @ W3 + b3)
where segsum(X)[i] = sum_{e: dst[e]=i} X[src[e]].

Reorder: segsum(X) @ W == segsum(X @ W), so aggregate in the (smaller)
output dim of each linear layer:
  h0 = feat @ W1            (token-major matmul, 20000x1433x1000)
  x1 = relu(segsum(h0)+b1)  (gather rows of h0 by src, segment-sum by dst)
  h1 = x1 @ W2
  x2 = relu(segsum(h1)+b2)
  out = relu(x2 @ W3 + b3)

Distribution: nodes row-sharded 2500/core. Each core computes h0/h1 for its
rows. The AllGather of h0/h1 is split into 4 row-chunk collectives (blocks
[0:5) [5:10) [10:15) [15:20) per core), each into its own Shared scratch
tensor (Tile requires a single writer per Shared tensor), so chunk r's
collective starts as soon as its rows are produced and overlaps the rest of
the compute. Each core then aggregates the edges whose dst it owns: one
dma_gather per (128-dst block, chunk tensor) over the block's unique src
rows, segment-sum via selection matmuls S.T @ G on the PE with host-built
0/1 S tiles.

Biases b1/b2 ride as a virtual graph edge: an extra row appended to the last
AllGather chunk holds b1/b2 and every dst block has one edge from that row,
so no separate bias matmuls are needed. b3 is a K=1 matmul in the tiny
output GEMM.

L1 is fused per dst-block: gather -> S-matmuls -> relu -> transpose -> W2
-> h1 rows, keeping SBUF small so gathers run several blocks ahead.
"""
import numpy as np
import ml_dtypes

import concourse.bass as bass
import concourse.bacc as bacc
import concourse.tile as tile
import concourse.mybir as mybir
from concourse import bass_utils

bf16 = ml_dtypes.bfloat16

NCORES = 8
N_NODES = 20000
N_EDGES = 200000
D_IN = 1433
KF = 1536           # padded feature dim (12 x 128)
H1 = 1024           # padded hidden1 (real 1000)
H2 = 512            # padded hidden2 (real 500)
DO = 7
R = N_NODES // NCORES          # 2500 rows per core
TB = [128] * 19 + [68]         # token/dst blocks per core (sum = 2500)
NB = len(TB)
TB_OFF = np.concatenate([[0], np.cumsum(TB)]).astype(int)
BIAS_ID = N_NODES              # virtual node id for the bias row

# AllGather row chunks: AGB blocks per chunk
AGB = 5
NCH = NB // AGB                # 4 chunks
CH_OFF = [int(TB_OFF[r * AGB]) for r in range(NCH + 1)]   # 0,640,1280,1920,2500
# rows each core contributes to chunk r (last chunk carries the bias row)
CH_SZ = [CH_OFF[r + 1] - CH_OFF[r] for r in range(NCH)]
CH_SZ[NCH - 1] += 1            # 640,640,640,581


def _chunk_and_row(g):
    """Global node id (or BIAS_ID) -> (chunk r, row within h0_all_r)."""
    g = np.asarray(g, np.int64)
    c = g // R
    l = g % R
    r = np.searchsorted(np.array(CH_OFF[1:-1]), l, side="right")
    row = c * np.array(CH_SZ)[r] + (l - np.array(CH_OFF[:-1])[r])
    bias = g == BIAS_ID
    r = np.where(bias, NCH - 1, r)
    row = np.where(bias, CH_SZ[NCH - 1] - 1, row)   # core 0's bias copy
    return r, row


def _host_prep(features, src, dst, W1, b1, W2, b2, W3, b3):
    """Build per-core staged arrays (all sharding/sorting/padding on host)."""
    feat = np.asarray(features, np.float32)
    src = np.asarray(src).astype(np.int64)
    dst = np.asarray(dst).astype(np.int64)

    featT = np.zeros((KF, N_NODES), np.float32)
    featT[:D_IN, :] = feat.T
    featT = featT.astype(bf16)

    W1p = np.zeros((KF, H1), np.float32)
    W1p[:D_IN, : W1.shape[1]] = W1
    W1p = W1p.astype(bf16)
    W2p = np.zeros((H1, H2), np.float32)
    W2p[: W2.shape[0], : W2.shape[1]] = W2
    W2p = W2p.astype(bf16)
    # W3 host-swizzled to [128, 4*DO] so the DMA is partition-contiguous
    W3p = np.zeros((H2, DO), np.float32)
    W3p[: W3.shape[0], :] = W3
    W3sw = np.zeros((128, (H2 // 128) * DO), np.float32)
    for j in range(H2 // 128):
        W3sw[:, j * DO : (j + 1) * DO] = W3p[j * 128 : (j + 1) * 128, :]
    W3sw = W3sw.astype(bf16)

    b1bf = np.zeros((1, H1), np.float32)
    b1bf[0, : b1.shape[0]] = b1
    b1bf = b1bf.astype(bf16)
    b2bf = np.zeros((1, H2), np.float32)
    b2bf[0, : b2.shape[0]] = b2
    b2bf = b2bf.astype(bf16)
    b3p = np.zeros((1, DO), np.float32)
    b3p[0, : b3.shape[0]] = b3

    ident = np.eye(128, dtype=bf16)

    # ---- edge prep: partition by dst owner, sort by dst; per block: unique
    # srcs + one virtual bias edge (BIAS_ID -> every dst in block), grouped
    # by AllGather chunk (gathers target per-chunk Shared tensors).
    owner = dst // R
    per_core = []   # [core][block] = (cnt_by_chunk, rows_by_chunk, col_of_uniq, row_of_uniq, inv2, d_all)
    for c in range(NCORES):
        sel = np.nonzero(owner == c)[0]
        d_loc = dst[sel] - c * R
        order = np.argsort(d_loc, kind="stable")
        sel = sel[order]
        d_loc = d_loc[order]
        s_glob = src[sel]
        blk_of = np.searchsorted(TB_OFF[1:], d_loc, side="right")
        blocks = []
        for b in range(NB):
            m = blk_of == b
            tb = TB[b]
            s_all = np.concatenate([s_glob[m], np.full(tb, BIAS_ID, np.int64)])
            d_all = np.concatenate([d_loc[m] - TB_OFF[b], np.arange(tb)])
            uniq, inv = np.unique(s_all, return_inverse=True)
            r_u, row_u = _chunk_and_row(uniq)
            perm = np.argsort(r_u, kind="stable")      # group uniq by chunk
            newpos = np.empty(len(uniq), np.int64)
            newpos[perm] = np.arange(len(uniq))
            inv2 = newpos[inv]
            cnt = [int(np.sum(r_u == r)) for r in range(NCH)]
            rows_by_chunk = [row_u[perm][r_u[perm] == r] for r in range(NCH)]
            blocks.append((cnt, rows_by_chunk, inv2, d_all))
        per_core.append(blocks)

    # uniform (over cores) idx counts per (block, chunk), multiple of 128
    N16 = [[0] * NCH for _ in range(NB)]
    for b in range(NB):
        for r in range(NCH):
            mx = max(per_core[c][b][0][r] for c in range(NCORES))
            N16[b][r] = -(-mx // 128) * 128
    # slot columns per (block, chunk) and per block
    KC = [[-(-N16[b][r] // 128) for r in range(NCH)] for b in range(NB)]
    K_blk = [sum(KC[b]) for b in range(NB)]
    KOFF = [np.concatenate([[0], np.cumsum(KC[b])]).astype(int) for b in range(NB)]
    TC = sum(K_blk)
    CI0 = np.concatenate([[0], np.cumsum(K_blk)]).astype(int)
    # idx-column offsets per (block, chunk)
    OFF16 = np.zeros((NB, NCH + 1), int)
    tot = 0
    for b in range(NB):
        for r in range(NCH):
            OFF16[b][r] = tot
            tot += N16[b][r] // 16
        OFF16[b][NCH] = tot
    TOT16 = tot

    src_idx_per_core = []
    s_sw_per_core = []
    for c in range(NCORES):
        idx_arr = np.zeros((128, TOT16), np.int16)
        s_f32 = np.zeros((TC, 128, 128), np.float32)
        for b in range(NB):
            cnt, rows_by_chunk, inv2, d_all = per_core[c][b]
            cnt_off = np.concatenate([[0], np.cumsum(cnt)]).astype(int)
            # uniq position j (chunk-grouped) -> slot (col, row)
            n_u = cnt_off[-1]
            j = np.arange(n_u)
            r_of_j = np.searchsorted(cnt_off[1:], j, side="right")
            j_in = j - cnt_off[r_of_j]
            col_of_j = KOFF[b][r_of_j] + j_in // 128
            row_of_j = j_in % 128
            # S tiles: S[chunk][slot, m] = count of edges (src slot, dst m)
            np.add.at(
                s_f32,
                (CI0[b] + col_of_j[inv2], row_of_j[inv2], d_all),
                1.0,
            )
            # gather idx per chunk, wrapped [16, n/16], x8 replicas; -1 pad
            for r in range(NCH):
                n = N16[b][r]
                if n == 0:
                    continue
                vals = np.zeros(n, np.int64)
                vals[: cnt[r]] = rows_by_chunk[r]
                wrapped = vals.reshape(-1, 16).T.astype(np.int16)
                idx_arr[:, OFF16[b][r] : OFF16[b][r + 1]] = np.tile(wrapped, (8, 1))
        src_idx_per_core.append(idx_arr)
        s_sw_per_core.append(np.ascontiguousarray(s_f32.astype(bf16).transpose(1, 0, 2)))

    static = dict(K_blk=K_blk, KC=KC, KOFF=KOFF, N16=N16, TC=TC, CI0=CI0,
                  OFF16=OFF16, TOT16=TOT16)
    shared = dict(W1p=W1p, W2p=W2p, W3sw=W3sw, b1bf=b1bf, b2bf=b2bf, b3p=b3p,
                  ident=ident)
    in_maps = []
    for c in range(NCORES):
        in_maps.append(
            dict(
                featT=np.ascontiguousarray(featT[:, c * R : (c + 1) * R]),
                src_idx=src_idx_per_core[c],
                s_sw=s_sw_per_core[c],
                **shared,
            )
        )
    return static, in_maps


def _build_program(static):
    K_blk, KC, KOFF, N16, TC, CI0, OFF16, TOT16 = (
        static["K_blk"], static["KC"], static["KOFF"], static["N16"],
        static["TC"], static["CI0"], static["OFF16"], static["TOT16"],
    )
    f32 = mybir.dt.float32
    b16 = mybir.dt.bfloat16
    i16 = mybir.dt.int16

    nc = bacc.Bacc(
        "TRN2", target_bir_lowering=False, debug=False,
        enable_asserts=False, num_devices=NCORES,
    )

    featT_d = nc.dram_tensor("featT", [KF, R], b16, kind="ExternalInput")
    W1_d = nc.dram_tensor("W1p", [KF, H1], b16, kind="ExternalInput")
    W2_d = nc.dram_tensor("W2p", [H1, H2], b16, kind="ExternalInput")
    W3_d = nc.dram_tensor("W3sw", [128, (H2 // 128) * DO], b16, kind="ExternalInput")
    b1_d = nc.dram_tensor("b1bf", [1, H1], b16, kind="ExternalInput")
    b2_d = nc.dram_tensor("b2bf", [1, H2], b16, kind="ExternalInput")
    b3_d = nc.dram_tensor("b3p", [1, DO], f32, kind="ExternalInput")
    id_d = nc.dram_tensor("ident", [128, 128], b16, kind="ExternalInput")
    idx_d = nc.dram_tensor("src_idx", [128, TOT16], i16, kind="ExternalInput")
    ssw_d = nc.dram_tensor("s_sw", [128, TC, 128], b16, kind="ExternalInput")
    out_d = nc.dram_tensor("out", [R, DO], f32, kind="ExternalOutput")

    with tile.TileContext(nc) as tc:
        with (
            tc.tile_pool(name="const", bufs=1) as constp,
            tc.tile_pool(name="dram", bufs=1, space="DRAM") as dram,
        ):
            # ---- constants
            idx_sb = constp.tile([128, TOT16], i16, name="c_idx", tag="idx")
            nc.sync.dma_start(idx_sb[:], idx_d.ap())
            ident = constp.tile([128, 128], b16, name="c_ident", tag="ident")
            nc.sync.dma_start(ident[:], id_d.ap())
            ones1 = constp.tile([1, 128], f32, name="c_ones1", tag="ones1")
            nc.vector.memset(ones1[:], 1.0)
            b3_sb = constp.tile([1, DO], f32, name="c_b3", tag="b3")
            nc.sync.dma_start(b3_sb[:], b3_d.ap())

            # ---- DRAM scratch. h0_in/h1_in carry the bias row at index R.
            h0_in = dram.tile([R + 1, H1], b16, name="h0in", tag="h0in")
            h1_in = dram.tile([R + 1, H2], b16, name="h1in", tag="h1in")
            h0_all = []
            h1_all = []
            for r in range(NCH):
                h0_all.append(dram.tile([NCORES * CH_SZ[r], H1], b16,
                                        name=f"h0all{r}", tag=f"h0all{r}",
                                        addr_space="Shared"))
                h1_all.append(dram.tile([NCORES * CH_SZ[r], H2], b16,
                                        name=f"h1all{r}", tag=f"h1all{r}",
                                        addr_space="Shared"))

            # bias rows into the AG inputs (local writes, same on every core);
            # staged through SBUF (DRAM->DRAM DMA is not reliable here)
            b1_sb = constp.tile([1, H1], b16, name="c_b1", tag="b1")
            nc.sync.dma_start(b1_sb[:], b1_d.ap())
            nc.sync.dma_start(h0_in[:][R : R + 1, :], b1_sb[:])
            b2_sb = constp.tile([1, H2], b16, name="c_b2", tag="b2")
            nc.sync.dma_start(b2_sb[:], b2_d.ap())
            nc.sync.dma_start(h1_in[:][R : R + 1, :], b2_sb[:])

            def ag_chunk(r, src_t, dst_t):
                lo = CH_OFF[r]
                hi = CH_OFF[r] + CH_SZ[r]     # last chunk includes bias row
                nc.gpsimd.collective_compute(
                    "AllGather", mybir.AluOpType.bypass,
                    replica_groups=[list(range(NCORES))],
                    ins=[src_t[:][lo:hi, :]],
                    outs=[dst_t[:]],
                )

            # ================= Phase A: h0 = featT.T @ W1 (token-major);
            # AllGather chunk r fires as soon as blocks [r*AGB,(r+1)*AGB) land.
            with (
                tc.tile_pool(name="featp", bufs=12) as featp,
                tc.tile_pool(name="w1p", bufs=12) as w1p,
                tc.tile_pool(name="h0out", bufs=4) as h0outp,
                tc.tile_pool(name="psA", bufs=8, space="PSUM") as psA,
            ):
                featc = []
                w1c = []
                for k in range(KF // 128):
                    ft = featp.tile([128, R], b16, name="a_featc", tag="featc")
                    nc.sync.dma_start(ft[:], featT_d.ap()[k * 128 : (k + 1) * 128, :])
                    featc.append(ft)
                    wt = w1p.tile([128, H1], b16, name="a_w1c", tag="w1c")
                    nc.sync.dma_start(wt[:], W1_d.ap()[k * 128 : (k + 1) * 128, :])
                    w1c.append(wt)
                nk = KF // 128
                for t in range(NB):
                    sl = slice(TB_OFF[t], TB_OFF[t + 1])
                    tb = TB[t]
                    ps = [psA.tile([128, 512], f32, name="a_psA", tag="psA")
                          for _ in range(2)]
                    for k in range(nk):
                        for h in range(2):
                            nc.tensor.matmul(
                                ps[h][:tb, :],
                                featc[k][:, sl],
                                w1c[k][:, h * 512 : (h + 1) * 512],
                                start=(k == 0),
                                stop=(k == nk - 1),
                            )
                    o = h0outp.tile([128, H1], b16, name="a_h0o", tag="h0o")
                    for h in range(2):
                        nc.vector.tensor_copy(o[:tb, h * 512 : (h + 1) * 512],
                                              ps[h][:tb, :])
                    nc.sync.dma_start(h0_in[:][sl, :], o[:tb, :])
                    if (t + 1) % AGB == 0:
                        ag_chunk(t // AGB, h0_in, h0_all[t // AGB])

            # ================= L1 (fused per block): gathers -> S-matmul agg
            # -> relu -> transpose -> W2 -> h1 rows; h1 AG chunks interleave.
            with (
                tc.tile_pool(name="gout", bufs=5) as goutp,
                tc.tile_pool(name="sp", bufs=4) as sp,
                tc.tile_pool(name="x1p", bufs=2) as x1p,
                tc.tile_pool(name="x1T", bufs=2 * (H1 // 128)) as x1Tp,
                tc.tile_pool(name="w2p", bufs=H1 // 128) as w2p,
                tc.tile_pool(name="h1o", bufs=3) as h1op,
                tc.tile_pool(name="psAgg", bufs=4, space="PSUM") as psAgg,
                tc.tile_pool(name="psTr", bufs=2, space="PSUM") as psTr,
                tc.tile_pool(name="psH1", bufs=2, space="PSUM") as psH1,
            ):
                w2c = []
                for j in range(H1 // 128):
                    wt = w2p.tile([128, H2], b16, name="l1_w2c", tag="w2c")
                    nc.sync.dma_start(wt[:], W2_d.ap()[j * 128 : (j + 1) * 128, :])
                    w2c.append(wt)
                kmax = max(K_blk)

                for b in range(NB):
                    kb = K_blk[b]
                    tb = TB[b]
                    sl = slice(TB_OFF[b], TB_OFF[b + 1])
                    g = goutp.tile([128, kmax, H1], b16, name="l1_g", tag="gout")
                    for r in range(NCH):
                        if N16[b][r] == 0:
                            continue
                        nc.gpsimd.dma_gather(
                            g[:, KOFF[b][r] : KOFF[b][r] + KC[b][r], :],
                            h0_all[r][:],
                            idx_sb[:, OFF16[b][r] : OFF16[b][r + 1]],
                            num_idxs=N16[b][r], num_idxs_reg=N16[b][r],
                            elem_size=H1, single_packet=False,
                        )
                    st = sp.tile([128, kmax, 128], b16, name="l1_st", tag="st")
                    nc.sync.dma_start(
                        st[:, :kb, :], ssw_d.ap()[:, CI0[b] : CI0[b + 1], :]
                    )
                    x1b = x1p.tile([128, H1], b16, name="l1_x1", tag="x1")
                    for h in range(2):
                        agg = psAgg.tile([128, 512], f32, name="l1_agg", tag="agg")
                        for k in range(kb):
                            nc.tensor.matmul(
                                agg[:], st[:, k, :], g[:, k, h * 512 : (h + 1) * 512],
                                start=(k == 0), stop=(k == kb - 1),
                            )
                        nc.vector.tensor_scalar_max(
                            x1b[:, h * 512 : (h + 1) * 512], agg[:], 0.0
                        )
                    # transpose 8 chunks, W2 matmul, h1 rows out
                    x1T = []
                    for j in range(H1 // 128):
                        trp = psTr.tile([128, 128], b16, name="l1_tr", tag="tr")
                        nc.tensor.transpose(
                            trp[:, :tb],
                            x1b[:tb, j * 128 : (j + 1) * 128],
                            ident[:tb, :tb],
                        )
                        xt = x1Tp.tile([128, 128], b16, name="l1_x1T", tag="x1T")
                        nc.vector.tensor_copy(xt[:, :tb], trp[:, :tb])
                        x1T.append(xt)
                    ph = psH1.tile([128, H2], f32, name="l1_psh1", tag="psh1")
                    nj = H1 // 128
                    for j in range(nj):
                        nc.tensor.matmul(
                            ph[:tb, :], x1T[j][:, :tb], w2c[j][:],
                            start=(j == 0), stop=(j == nj - 1),
                        )
                    ho = h1op.tile([128, H2], b16, name="l1_h1o", tag="h1o")
                    nc.vector.tensor_copy(ho[:tb, :], ph[:tb, :])
                    nc.sync.dma_start(h1_in[:][sl, :], ho[:tb, :])
                    if (b + 1) % AGB == 0:
                        ag_chunk(b // AGB, h1_in, h1_all[b // AGB])

            # ================= L2 (fused per block): gathers -> S-matmul agg
            # -> relu -> transpose -> W3 (+b3) -> out rows
            with (
                tc.tile_pool(name="gout2", bufs=8) as goutp2,
                tc.tile_pool(name="sp2", bufs=4) as sp2,
                tc.tile_pool(name="x2p", bufs=2) as x2p,
                tc.tile_pool(name="x2T", bufs=2 * (H2 // 128)) as x2Tp,
                tc.tile_pool(name="w3p", bufs=1) as w3p,
                tc.tile_pool(name="outp", bufs=3) as outp,
                tc.tile_pool(name="psAgg2", bufs=4, space="PSUM") as psAgg2,
                tc.tile_pool(name="psTr2", bufs=2, space="PSUM") as psTr2,
                tc.tile_pool(name="psO", bufs=2, space="PSUM") as psO,
            ):
                w3t = w3p.tile([128, (H2 // 128) * DO], b16, tag="w3")
                nc.sync.dma_start(w3t[:], W3_d.ap())
                kmax = max(K_blk)

                for b in range(NB):
                    kb = K_blk[b]
                    tb = TB[b]
                    sl = slice(TB_OFF[b], TB_OFF[b + 1])
                    g = goutp2.tile([128, kmax, H2], b16, name="l2_g", tag="gout2")
                    for r in range(NCH):
                        if N16[b][r] == 0:
                            continue
                        nc.gpsimd.dma_gather(
                            g[:, KOFF[b][r] : KOFF[b][r] + KC[b][r], :],
                            h1_all[r][:],
                            idx_sb[:, OFF16[b][r] : OFF16[b][r + 1]],
                            num_idxs=N16[b][r], num_idxs_reg=N16[b][r],
                            elem_size=H2, single_packet=False,
                        )
                    st = sp2.tile([128, kmax, 128], b16, name="l2_st", tag="st2")
                    nc.sync.dma_start(
                        st[:, :kb, :], ssw_d.ap()[:, CI0[b] : CI0[b + 1], :]
                    )
                    agg = psAgg2.tile([128, H2], f32, name="l2_agg", tag="agg2")
                    for k in range(kb):
                        nc.tensor.matmul(
                            agg[:], st[:, k, :], g[:, k, :],
                            start=(k == 0), stop=(k == kb - 1),
                        )
                    x2b = x2p.tile([128, H2], b16, name="l2_x2", tag="x2")
                    nc.vector.tensor_scalar_max(x2b[:], agg[:], 0.0)
                    x2T = []
                    for j in range(H2 // 128):
                        trp = psTr2.tile([128, 128], b16, name="l2_tr", tag="tr2")
                        nc.tensor.transpose(
                            trp[:, :tb],
                            x2b[:tb, j * 128 : (j + 1) * 128],
                            ident[:tb, :tb],
                        )
                        xt = x2Tp.tile([128, 128], b16, name="l2_x2T", tag="x2T")
                        nc.vector.tensor_copy(xt[:, :tb], trp[:, :tb])
                        x2T.append(xt)
                    po = psO.tile([128, DO], f32, name="l2_pso", tag="pso")
                    nj = H2 // 128
                    for j in range(nj):
                        nc.tensor.matmul(
                            po[:tb, :], x2T[j][:, :tb],
                            w3t[:, j * DO : (j + 1) * DO],
                            start=(j == 0), stop=False,
                        )
                    nc.tensor.matmul(po[:tb, :], ones1[:, :tb], b3_sb[:],
                                     start=False, stop=True)
                    oo = outp.tile([128, DO], f32, name="l2_oo", tag="oo")
                    nc.vector.tensor_scalar_max(oo[:tb, :], po[:tb, :], 0.0)
                    nc.sync.dma_start(out_d.ap()[sl, :], oo[:tb, :])

    nc.compile()
    return nc


def kernel_with_results(features, src, dst, W1, b1, W2, b2, W3, b3, trace=False,
                        stage="full"):
    static, in_maps = _host_prep(features, src, dst, W1, b1, W2, b2, W3, b3)
    nc = _build_program(static)
    res = bass_utils.run_bass_kernel_spmd(
        nc, in_maps, core_ids=list(range(NCORES)), trace=trace
    )
    out = np.concatenate([res.results[c]["out"] for c in range(NCORES)], axis=0)
    return out.astype(np.float32), res


def kernel(features, src, dst, W1, b1, W2, b2, W3, b3):
    out, _ = kernel_with_results(features, src, dst, W1, b1, W2, b2, W3, b3)
    return out
